# revision 3
# baseline (speedup 1.0000x reference)
"""Trainium2 Bass kernel for gnn_message_passing (nn_FGL_2138893714004).

Reference computation:
    y = x * nf_weight                    # (8, 32, 50000)
    g = y[:, :, A]                       # (8, 32, 8192, 32)
    red = max(g, axis=-1)                # (8, 32, 8192)
    out = einsum('nio,ik->nko', red, ft) # (8, 64, 8192)
    out = out + bias                     # bias (64, 8192)

Strategy (8 NeuronCores): shard the 8192 output nodes 8 ways (1024 per
core).  The host stages y = x * nf_weight token-major (rows[j] =
y[:, :, j].ravel(), 256 bf16 = 512 B) and sends each core a compacted
table of only the rows its shard references (np.unique, ~24k distinct
rows < 2^15 so indices fit dma_gather's int16).

On-core pipeline, one o-quarter (256 nodes) at a time, two gathers per
quarter (16 of 32 sorted neighbor slots each):

  prepare_only SWDGE dma_gather + trigger_dma  -- desc-gen on the Pool
      engine overlaps the previous gather's SDMA transfer instead of
      blocking the engine until completion (the old blocking form
      serialized 8 x 27 us of gather instructions back-to-back);
  DVE pairwise max tree folds 16 neighbor slots -> 1 in place;
  tail per quarter (overlapped with the next quarter's gathers):
      PE transposes red to channel-major, 2-batch block-diagonal
      128x128 matmuls against ft_weight (4 instead of 16 per quarter),
      DVE bias add straight out of PSUM, bf16 store.
"""

import sys

sys.path.insert(0, "/opt/trn_rl_repo")

import ml_dtypes
import numpy as np

import concourse.bacc as bacc
import concourse.mybir as mybir
from concourse.bass_utils import run_bass_kernel_spmd
from concourse.masks import make_identity
from concourse.tile import TileContext

N, INC, INN = 8, 32, 50000
OUTC, OUTN, D = 64, 8192, 32
NCORES = 8
O_SH = OUTN // NCORES          # 1024 output nodes per core
QNODES = 256                   # nodes per tail-pipelined quarter
NQUART = O_SH // QNODES        # 4 quarters
NG = NQUART * 2                # 8 gathers: (quarter, 16-of-32 d-slots)
NIDX = 16 * QNODES             # 4096 refs per gather
ROWE = N * INC                 # 256 bf16 per table row (512 B)
VCAP = 32768                   # compacted table capacity (int16 range)
NQ = 4                         # SWDGE queues (ucode max)
BF16 = mybir.dt.bfloat16
FP32 = mybir.dt.float32

_cache: dict = {}


def _build(reps: int = 1, stages: str = 'full', nq: int = NQ, gb: int = 3):
    nc = bacc.Bacc("TRN2", target_bir_lowering=False, debug=False,
                   num_devices=NCORES, num_swdge_queues=nq)
    tab = nc.dram_tensor("tab", [VCAP, ROWE], BF16, kind="ExternalInput")
    idx = nc.dram_tensor("idx", [128, NG, NIDX // 16], mybir.dt.int16,
                         kind="ExternalInput")
    bd = nc.dram_tensor("bd", [128, 2, 128], BF16, kind="ExternalInput")
    bias2 = nc.dram_tensor("bias2", [128, O_SH], FP32, kind="ExternalInput")
    out = nc.dram_tensor("out", [N, OUTC, O_SH], BF16, kind="ExternalOutput")

    with TileContext(nc) as tc:
        with (
            tc.tile_pool(name="persist", bufs=1) as pp,
            tc.tile_pool(name="g", bufs=gb) as gp,
            tc.tile_pool(name="red", bufs=2) as rp,
            tc.tile_pool(name="rt", bufs=2) as rtp,
            tc.tile_pool(name="outs", bufs=4) as op,
            tc.tile_pool(name="pst", bufs=2, space="PSUM") as pstp,
            tc.tile_pool(name="psm", bufs=2, space="PSUM") as psmp,
        ):
            idx_sb = pp.tile([128, NG, NIDX // 16], mybir.dt.int16)
            nc.sync.dma_start(out=idx_sb[:], in_=idx[:, :, :])
            bd_sb = pp.tile([128, 2, 128], BF16)
            nc.sync.dma_start(out=bd_sb[:], in_=bd[:, :, :])
            bias_sb = pp.tile([128, O_SH], FP32)
            nc.sync.dma_start(out=bias_sb[:], in_=bias2[:, :])
            ident = pp.tile([128, 128], BF16)
            make_identity(nc, ident[:])

            sems = [nc.alloc_semaphore(f"gsem{i}") for i in range(NG)]
            nums = sorted(s.num for s in sems)
            assert nums == list(range(nums[0], nums[0] + NG))
            srange = range(nums[0], nums[-1] + 1)
            nc.gpsimd.dma_reset(srange)
            nc.gpsimd.sem_clear(srange)

            for _rep in range(reps):
              for q in range(NQUART):
                # running max over the quarter: [p, ohi, n, ch]
                red = rp.tile([128, 2, N, INC], BF16, tag="red")
                rflat = red[:].rearrange("p a b c -> p (a b c)")
                for h in range(2):
                    gi = q * 2 + h
                    g = gp.tile([128, 32, ROWE], BF16, tag="g")
                    if stages != 'compute':
                        nc.gpsimd.dma_gather(
                            g[:], tab[:, :], idx_sb[:, gi, :],
                            NIDX, NIDX, ROWE,
                            single_packet=False,
                            queue_num=gi % nq,
                            prepare_only=True,
                            sem=sems[gi],
                        )
                        nc.gpsimd.trigger_dma(count=None, queue_num=gi % nq)
                    else:
                        nc.vector.memset(g[:, 0:1, 0:1], 0.0)
                    if stages == 'gather':
                        continue
                    # fold 16 d-slots: rows (ds, ohi) -> [p, 16, 512]
                    v = g[:].rearrange("p (a b) e -> p a (b e)", a=16)
                    nc.vector.tensor_tensor(out=v[:, 0:8, :], in0=v[:, 0:8, :],
                                            in1=v[:, 8:16, :],
                                            op=mybir.AluOpType.max)
                    nc.vector.tensor_tensor(out=v[:, 0:4, :], in0=v[:, 0:4, :],
                                            in1=v[:, 4:8, :],
                                            op=mybir.AluOpType.max)
                    nc.vector.tensor_tensor(out=v[:, 0:2, :], in0=v[:, 0:2, :],
                                            in1=v[:, 2:4, :],
                                            op=mybir.AluOpType.max)
                    if h == 0:
                        nc.vector.tensor_tensor(out=rflat, in0=v[:, 0, :],
                                                in1=v[:, 1, :],
                                                op=mybir.AluOpType.max)
                    else:
                        nc.vector.tensor_tensor(out=v[:, 0, :], in0=v[:, 0, :],
                                                in1=v[:, 1, :],
                                                op=mybir.AluOpType.max)
                        nc.vector.tensor_tensor(out=rflat, in0=rflat,
                                                in1=v[:, 0, :],
                                                op=mybir.AluOpType.max)

                if stages in ('gather', 'nogather_notail', 'gather_mulfold'):
                    continue
                # ---- tail for this quarter ----
                # red flat cols = (ohi, n, ch); transpose 128-col blocks
                # t = (ohi, nh) to rt[nh] cols (ohi, node).
                rts = [rtp.tile([128, 2, 128], BF16, tag=f"rt{nh}",
                                name=f"rt{nh}")
                       for nh in range(2)]
                for t in range(4):
                    ohi, nh = t // 2, t % 2
                    pst = pstp.tile([128, 128], BF16, tag="pst")
                    nc.tensor.transpose(
                        out=pst[:],
                        in_=rflat[:, t * 128:(t + 1) * 128],
                        identity=ident[:],
                    )
                    nc.vector.tensor_copy(out=rts[nh][:, ohi, :], in_=pst[:])

                # 2-batch block-diag matmuls: pair pi covers batches
                # (2*pi, 2*pi+1); rhs = rt[nh] flat [128, 256].
                for pi in range(4):
                    nh, bdi = pi // 2, pi % 2
                    pso = psmp.tile([128, QNODES], FP32, tag="pso")
                    nc.tensor.matmul(
                        out=pso[:],
                        lhsT=bd_sb[:, bdi, :],
                        rhs=rts[nh][:].rearrange("p a b -> p (a b)"),
                        start=True, stop=True,
                    )
                    osb = op.tile([128, QNODES], BF16, tag="osb")
                    nc.vector.tensor_tensor(
                        out=osb[:], in0=pso[:],
                        in1=bias_sb[:, q * QNODES:(q + 1) * QNODES],
                        op=mybir.AluOpType.add)
                    ne = 2 * pi
                    nc.sync.dma_start(
                        out=out[ne, :, q * QNODES:(q + 1) * QNODES],
                        in_=osb[0:OUTC, :])
                    nc.sync.dma_start(
                        out=out[ne + 1, :, q * QNODES:(q + 1) * QNODES],
                        in_=osb[OUTC:128, :])

    nc.compile()
    return nc


def _prep(x, nf_weight, ft_weight, bias, A):
    bf = ml_dtypes.bfloat16
    # token-major y rows: rows[j] = (x * nf)[:, :, j].ravel()
    y = x * nf_weight[None]
    rows = np.ascontiguousarray(y.transpose(2, 0, 1)).reshape(
        INN, ROWE).astype(bf)

    ftb = ft_weight.astype(bf)
    bdm = np.zeros((128, 2, 128), dtype=bf)
    bdm[0:32, 0, 0:64] = ftb
    bdm[32:64, 0, 64:128] = ftb
    bdm[64:96, 1, 0:64] = ftb
    bdm[96:128, 1, 64:128] = ftb

    in_maps = []
    for s in range(NCORES):
        A_s = np.sort(A[s * O_SH:(s + 1) * O_SH], axis=1)  # (1024, 32)
        uniq, inv = np.unique(A_s, return_inverse=True)
        assert len(uniq) <= VCAP, len(uniq)
        tab = np.zeros((VCAP, ROWE), dtype=bf)
        tab[:len(uniq)] = rows[uniq]
        remap = inv.reshape(A_s.shape).astype(np.int16)  # [o_loc, d]
        idx16 = np.zeros((128, NG, NIDX // 16), dtype=np.int16)
        for gi in range(NG):
            q, h = gi // 2, gi % 2
            sub = remap[q * QNODES:(q + 1) * QNODES,
                        h * 16:(h + 1) * 16]      # [256 nodes, 16 ds]
            flat = sub.T.reshape(-1)              # j = ds*256 + o_loc
            idx16[:16, gi, :] = flat.reshape(NIDX // 16, 16).T
        idx16[16:] = np.tile(idx16[:16], (7, 1, 1))
        bias_sh = bias[:, s * O_SH:(s + 1) * O_SH].astype(np.float32)
        in_maps.append({
            "tab": tab,
            "idx": idx16,
            "bd": bdm,
            "bias2": np.ascontiguousarray(np.tile(bias_sh, (2, 1))),
        })
    return in_maps


def run(x, nf_weight, ft_weight, bias, A, reps=1, stages='full', **run_kwargs):
    """Build (cached), run on 8 cores, reassemble. Returns (out, results)."""
    key = ("nc", reps, stages)
    if key not in _cache:
        _cache[key] = _build(reps, stages)
    nc = _cache[key]
    in_maps = _prep(np.asarray(x), np.asarray(nf_weight),
                    np.asarray(ft_weight), np.asarray(bias), np.asarray(A))
    res = run_bass_kernel_spmd(nc, in_maps, core_ids=list(range(NCORES)),
                               **run_kwargs)
    out = np.empty((N, OUTC, OUTN), dtype=np.float32)
    for s in range(NCORES):
        out[:, :, s * O_SH:(s + 1) * O_SH] = res.results[s]["out"].astype(
            np.float32)
    return out, res


def kernel(x, nf_weight, ft_weight, bias, A):
    out, _ = run(x, nf_weight, ft_weight, bias, A)
    return out


# revision 7
# speedup vs baseline: 2.4772x; 2.4772x over previous
"""Trainium2 Bass kernel for gnn_message_passing (nn_FGL_2138893714004).

Reference computation:
    y = x * nf_weight                    # (8, 32, 50000)
    g = y[:, :, A]                       # (8, 32, 8192, 32)
    red = max(g, axis=-1)                # (8, 32, 8192)
    out = einsum('nio,ik->nko', red, ft) # (8, 64, 8192)
    out = out + bias                     # bias (64, 8192)

Strategy (8 NeuronCores): shard the 8192 output nodes 8 ways (1024 per
core).  The host stages y = x * nf_weight token-major and packs each
core's gather payload as GROUP=4 consecutive sorted neighbors per table
row (2 KB rows, deduplicated with np.unique; row indices fit
dma_gather's int16).  Grouping matters because SWDGE descriptor
generation on the Pool engine costs ~8 ns/descriptor on hardware --
with 512 B single-token rows the 32768 descriptors/core would pin the
Pool engine for ~250 us, far above the ~55 us HBM transfer time of the
payload.  At 2 KB/descriptor the desc-gen (~8192 descs, ~70 us)
pipelines against the transfer.

On-core pipeline, one o-quarter (256 nodes) at a time, two gathers per
quarter:

  dma_gather(prepare_only) + trigger_dma: desc-gen on Pool overlaps the
      previous gather's SDMA transfer (a blocking gather holds the Pool
      engine through both desc-gen and transfer, serializing
      everything);
  explicit wait_ge on the descriptor-baked DMA-completion semaphore
      gates the DVE consumers (Tile's automatic dep points at the
      prep's desc-write tick, which is too early);
  DVE pairwise-max tree folds 4 tokens/row then 4 slots then 2 halves;
  tail per quarter, overlapped with the next quarter's gathers:
      PE transposes red to channel-major, 2-batch block-diagonal
      128x128 matmuls against ft_weight, DVE bias add from PSUM,
      bf16 store (host casts back to float32).
"""

import sys

sys.path.insert(0, "/opt/trn_rl_repo")

import ml_dtypes
import numpy as np

import concourse.bacc as bacc
import concourse.mybir as mybir
from concourse.bass_utils import run_bass_kernel_spmd
from concourse.masks import make_identity
from concourse.tile import TileContext

N, INC, INN = 8, 32, 50000
OUTC, OUTN, D = 64, 8192, 32
NCORES = 8
O_SH = OUTN // NCORES          # 1024 output nodes per core
QNODES = 256                   # nodes per tail-pipelined quarter
NQUART = O_SH // QNODES        # 4 quarters
GROUP = 4                      # tokens per table row
NSLOT = D // GROUP             # 8 group-slots per node
GPQ = 2                        # gathers per quarter
SPG = NSLOT // GPQ             # 4 group-slots per gather
NIDX = SPG * QNODES            # 1024 refs per gather
NGATH = NQUART * GPQ           # 8 gathers per core
RPG = NIDX // 128              # 8 sbuf rows per gather
TOKE = N * INC                 # 256 elems per token
ROWE = GROUP * TOKE            # row elems (bf16)
VCAPG = (O_SH * D) // GROUP    # 8192 group rows max per core
NQ = 4                         # SWDGE queues (ucode max)
BF16 = mybir.dt.bfloat16
FP32 = mybir.dt.float32
MAX = mybir.AluOpType.max

_cache: dict = {}


def _build(reps: int = 1, stages: str = 'full', nq: int = NQ, gb: int = 3):
    nc = bacc.Bacc("TRN2", target_bir_lowering=False, debug=False,
                   num_devices=NCORES, num_swdge_queues=nq)
    tab = nc.dram_tensor("tab", [VCAPG, ROWE], BF16, kind="ExternalInput")
    idx = nc.dram_tensor("idx", [128, NGATH, NIDX // 16], mybir.dt.int16,
                         kind="ExternalInput")
    bd = nc.dram_tensor("bd", [128, 2, 128], BF16, kind="ExternalInput")
    bias2 = nc.dram_tensor("bias2", [128, O_SH], FP32, kind="ExternalInput")
    out = nc.dram_tensor("out", [N, OUTC, O_SH], BF16, kind="ExternalOutput")

    with TileContext(nc) as tc:
        with (
            tc.tile_pool(name="persist", bufs=1) as pp,
            tc.tile_pool(name="g", bufs=gb) as gp,
            tc.tile_pool(name="red", bufs=2) as rp,
            tc.tile_pool(name="rt", bufs=2) as rtp,
            tc.tile_pool(name="outs", bufs=4) as op,
            tc.tile_pool(name="pst", bufs=2, space="PSUM") as pstp,
            tc.tile_pool(name="psm", bufs=2, space="PSUM") as psmp,
        ):
            idx_sb = pp.tile([128, NGATH, NIDX // 16], mybir.dt.int16)
            nc.sync.dma_start(out=idx_sb[:], in_=idx[:, :, :])
            bd_sb = pp.tile([128, 2, 128], BF16)
            nc.sync.dma_start(out=bd_sb[:], in_=bd[:, :, :])
            bias_sb = pp.tile([128, O_SH], FP32)
            nc.sync.dma_start(out=bias_sb[:], in_=bias2[:, :])
            ident = pp.tile([128, 128], BF16)
            make_identity(nc, ident[:])

            sems = [nc.alloc_semaphore(f"gsem{i}") for i in range(NGATH)]
            nums = sorted(s.num for s in sems)
            assert nums == list(range(nums[0], nums[0] + NGATH))
            srange = range(nums[0], nums[-1] + 1)
            nc.gpsimd.dma_reset(srange)
            nc.gpsimd.sem_clear(srange)

            for rep in range(reps):
              for q in range(NQUART):
                # running max over the quarter: [p, ohi, (n, ch)]
                red = rp.tile([128, 2, TOKE], BF16, tag="red")
                redf = red[:].rearrange("p a b -> p (a b)")
                for h in range(GPQ):
                    gi = q * GPQ + h
                    g = gp.tile([128, RPG, ROWE], BF16, tag="g")
                    if stages != 'compute':
                        nc.gpsimd.dma_gather(
                            g[:], tab[:, :], idx_sb[:, gi, :],
                            NIDX, NIDX, ROWE,
                            single_packet=False,
                            queue_num=gi % nq,
                            prepare_only=True,
                            sem=sems[gi],
                        )
                        nc.gpsimd.trigger_dma(count=None, queue_num=gi % nq)
                        if rep == 0 and stages != 'nowait':
                            nc.vector.wait_ge(sems[gi], 16)
                    else:
                        nc.vector.memset(g[:, 0:1, 0:1], 0.0)
                    if stages == 'gather':
                        continue
                    # fold GROUP tokens within each row: [p, r, t, e]
                    g4 = g[:].rearrange("p r (t e) -> p r t e", t=GROUP)
                    t = GROUP
                    while t > 1:
                        nc.vector.tensor_tensor(
                            out=g4[:, :, 0:t // 2, :],
                            in0=g4[:, :, 0:t // 2, :],
                            in1=g4[:, :, t // 2:t, :], op=MAX)
                        t //= 2
                    # fold SPG slots: rows r = (slot, ohi)
                    w = g4[:, :, 0, :].rearrange("p (a b) e -> p a b e",
                                                 a=SPG)
                    a = SPG
                    while a > 1:
                        nc.vector.tensor_tensor(
                            out=w[:, 0:a // 2], in0=w[:, 0:a // 2],
                            in1=w[:, a // 2:a], op=MAX)
                        a //= 2
                    # w[:, 0] = [p, ohi, e] quarter partial (strided)
                    if h == 0:
                        nc.vector.tensor_copy(out=red[:], in_=w[:, 0])
                    else:
                        nc.vector.tensor_tensor(out=red[:], in0=red[:],
                                                in1=w[:, 0], op=MAX)

                if stages in ('gather', 'nogather_notail'):
                    continue
                # ---- tail for this quarter ----
                # red cols = (ohi, n, ch); transpose 128-col blocks
                # t = (ohi, nh) into rt[nh] cols (ohi, node).
                rts = [rtp.tile([128, 2, 128], BF16, tag=f"rt{nh}",
                                name=f"rt{nh}")
                       for nh in range(2)]
                for tb in range(4):
                    ohi, nh = tb // 2, tb % 2
                    pst = pstp.tile([128, 128], BF16, tag="pst")
                    nc.tensor.transpose(
                        out=pst[:],
                        in_=redf[:, tb * 128:(tb + 1) * 128],
                        identity=ident[:],
                    )
                    nc.vector.tensor_copy(out=rts[nh][:, ohi, :], in_=pst[:])

                # 2-batch block-diag matmuls: pair pi covers batches
                # (2*pi, 2*pi+1); rhs = rt[nh] flat [128, 256].
                for pi in range(4):
                    nh, bdi = pi // 2, pi % 2
                    pso = psmp.tile([128, QNODES], FP32, tag="pso")
                    nc.tensor.matmul(
                        out=pso[:],
                        lhsT=bd_sb[:, bdi, :],
                        rhs=rts[nh][:].rearrange("p a b -> p (a b)"),
                        start=True, stop=True,
                    )
                    osb = op.tile([128, QNODES], BF16, tag="osb")
                    nc.vector.tensor_tensor(
                        out=osb[:], in0=pso[:],
                        in1=bias_sb[:, q * QNODES:(q + 1) * QNODES],
                        op=mybir.AluOpType.add)
                    ne = 2 * pi
                    nc.sync.dma_start(
                        out=out[ne, :, q * QNODES:(q + 1) * QNODES],
                        in_=osb[0:OUTC, :])
                    nc.sync.dma_start(
                        out=out[ne + 1, :, q * QNODES:(q + 1) * QNODES],
                        in_=osb[OUTC:128, :])

    nc.compile()
    return nc


def _prep(x, nf_weight, ft_weight, bias, A):
    bf = ml_dtypes.bfloat16
    # token-major y rows: rows[j] = (x * nf)[:, :, j].ravel()
    y = x * nf_weight[None]
    rows = np.ascontiguousarray(y.transpose(2, 0, 1)).reshape(
        INN, TOKE).astype(bf)

    ftb = ft_weight.astype(bf)
    bdm = np.zeros((128, 2, 128), dtype=bf)
    bdm[0:32, 0, 0:64] = ftb
    bdm[32:64, 0, 64:128] = ftb
    bdm[64:96, 1, 0:64] = ftb
    bdm[96:128, 1, 64:128] = ftb

    in_maps = []
    for s in range(NCORES):
        A_s = np.sort(A[s * O_SH:(s + 1) * O_SH], axis=1)  # (1024, 32)
        grp = A_s.reshape(O_SH * NSLOT, GROUP)             # group rows
        uniq, inv = np.unique(grp, axis=0, return_inverse=True)
        assert len(uniq) <= VCAPG, len(uniq)
        tab = np.zeros((VCAPG, ROWE), dtype=bf)
        tab[:len(uniq)] = rows[uniq].reshape(len(uniq), ROWE)
        remap = inv.reshape(O_SH, NSLOT).astype(np.int16)  # [o_loc, slot]
        idx16 = np.zeros((128, NGATH, NIDX // 16), dtype=np.int16)
        for gi in range(NGATH):
            q, h = gi // GPQ, gi % GPQ
            sub = remap[q * QNODES:(q + 1) * QNODES,
                        h * SPG:(h + 1) * SPG]    # [256 nodes, SPG slots]
            flat = sub.T.reshape(-1)              # j = slot*256 + o_loc
            idx16[:16, gi, :] = flat.reshape(NIDX // 16, 16).T
        idx16[16:] = np.tile(idx16[:16], (7, 1, 1))
        bias_sh = bias[:, s * O_SH:(s + 1) * O_SH].astype(np.float32)
        in_maps.append({
            "tab": tab,
            "idx": idx16,
            "bd": bdm,
            "bias2": np.ascontiguousarray(np.tile(bias_sh, (2, 1))),
        })
    return in_maps


def run(x, nf_weight, ft_weight, bias, A, reps=1, stages='full', **run_kwargs):
    """Build (cached), run on 8 cores, reassemble. Returns (out, results)."""
    key = ("nc", reps, stages)
    if key not in _cache:
        _cache[key] = _build(reps, stages)
    nc = _cache[key]
    in_maps = _prep(np.asarray(x), np.asarray(nf_weight),
                    np.asarray(ft_weight), np.asarray(bias), np.asarray(A))
    res = run_bass_kernel_spmd(nc, in_maps, core_ids=list(range(NCORES)),
                               **run_kwargs)
    out = np.empty((N, OUTC, OUTN), dtype=np.float32)
    for s in range(NCORES):
        out[:, :, s * O_SH:(s + 1) * O_SH] = res.results[s]["out"].astype(
            np.float32)
    return out, res


def kernel(x, nf_weight, ft_weight, bias, A):
    out, _ = run(x, nf_weight, ft_weight, bias, A)
    return out


# revision 15
# speedup vs baseline: 3.3224x; 1.3412x over previous
"""Trainium2 Bass kernel for gnn_message_passing (nn_FGL_2138893714004).

Reference computation:
    y = x * nf_weight                    # (8, 32, 50000)
    g = y[:, :, A]                       # (8, 32, 8192, 32)
    red = max(g, axis=-1)                # (8, 32, 8192)
    out = einsum('nio,ik->nko', red, ft) # (8, 64, 8192)
    out = out + bias                     # bias (64, 8192)

Strategy (8 NeuronCores): shard the 8192 output nodes 8 ways (1024 per
core).  The host stages y = x * nf_weight token-major and packs each
core's gather payload as GROUP=8 consecutive sorted neighbors per table
row (4 KB rows, deduplicated with np.unique; row indices fit
dma_gather's int16).  Grouping matters because SWDGE descriptor
generation on the Pool engine costs ~8 ns/descriptor on hardware --
with 512 B single-token rows the 32768 descriptors/core would pin the
Pool engine for ~250 us, far above the ~55 us HBM transfer time of the
payload.  At 4 KB/descriptor the desc-gen (4096 descs over 8 preps)
pipelines well under the transfer.

On-core pipeline, one o-quarter (256 nodes) at a time, two gathers per
quarter:

  dma_gather(prepare_only) + trigger_dma: desc-gen on Pool overlaps the
      previous gather's SDMA transfer (a blocking gather holds the Pool
      engine through both desc-gen and transfer, serializing
      everything);
  explicit wait_ge on the descriptor-baked DMA-completion semaphore
      gates the DVE consumers (Tile's automatic dep points at the
      prep's desc-write tick, which is too early);
  DVE pairwise-max tree folds 4 tokens/row then 4 slots then 2 halves;
  tail per quarter, overlapped with the next quarter's gathers:
      PE transposes red to channel-major, 2-batch block-diagonal
      128x128 matmuls against ft_weight, DVE bias add from PSUM,
      bf16 store (host casts back to float32).
"""

import sys

sys.path.insert(0, "/opt/trn_rl_repo")

import ml_dtypes
import numpy as np

import concourse.bacc as bacc
import concourse.mybir as mybir
from concourse.bass_utils import run_bass_kernel_spmd
from concourse.masks import make_identity
from concourse.tile import TileContext

N, INC, INN = 8, 32, 50000
OUTC, OUTN, D = 64, 8192, 32
NCORES = 8
O_SH = OUTN // NCORES          # 1024 output nodes per core
QNODES = 256                   # nodes per tail-pipelined quarter
NQUART = O_SH // QNODES        # 4 quarters
GROUP = 8                      # tokens per table row
NSLOT = D // GROUP             # group-slots per node
GPQ = 2                        # gathers per quarter
SPG = NSLOT // GPQ             # group-slots per gather
NIDX = SPG * QNODES            # 1024 refs per gather
NGATH = NQUART * GPQ           # 8 gathers per core
RPG = NIDX // 128              # 8 sbuf rows per gather
TOKE = N * INC                 # 256 elems per token
ROWE = GROUP * TOKE            # row elems (bf16)
VCAPG = (O_SH * D) // GROUP    # 8192 group rows max per core
NQ = 4                         # SWDGE queues (ucode max)
BF16 = mybir.dt.bfloat16
FP32 = mybir.dt.float32
MAX = mybir.AluOpType.max

_cache: dict = {}


def _build(reps: int = 1, stages: str = 'full', nq: int = NQ, gb: int = 3):
    nc = bacc.Bacc("TRN2", target_bir_lowering=False, debug=False,
                   num_devices=NCORES, num_swdge_queues=nq)
    tab = nc.dram_tensor("tab", [VCAPG, ROWE], BF16, kind="ExternalInput")
    idx = nc.dram_tensor("idx", [NGATH, 128, NIDX // 16], mybir.dt.int16,
                         kind="ExternalInput")
    bd = nc.dram_tensor("bd", [128, 2, 128], BF16, kind="ExternalInput")
    bias2 = nc.dram_tensor("bias2", [128, O_SH], FP32, kind="ExternalInput")
    out = nc.dram_tensor("out", [N, OUTC, O_SH], BF16, kind="ExternalOutput")

    with TileContext(nc) as tc:
        with (
            tc.tile_pool(name="persist", bufs=1) as pp,
            tc.tile_pool(name="g", bufs=gb) as gp,
            tc.tile_pool(name="red", bufs=2) as rp,
            tc.tile_pool(name="rt", bufs=2) as rtp,
            tc.tile_pool(name="outs", bufs=4) as op,
            tc.tile_pool(name="pst", bufs=2, space="PSUM") as pstp,
            tc.tile_pool(name="psm", bufs=2, space="PSUM") as psmp,
        ):
            # per-gather idx loads so prep 0 is unblocked after ~1 us
            idx_sb = pp.tile([128, NGATH, NIDX // 16], mybir.dt.int16)
            for gi in range(NGATH):
                nc.sync.dma_start(out=idx_sb[:, gi, :], in_=idx[gi, :, :])
            bd_sb = pp.tile([128, 2, 128], BF16)
            nc.sync.dma_start(out=bd_sb[:], in_=bd[:, :, :])
            bias_sb = pp.tile([128, O_SH], FP32)
            nc.sync.dma_start(out=bias_sb[:], in_=bias2[:, :])
            ident = pp.tile([128, 128], BF16)
            make_identity(nc, ident[:])

            sems = [nc.alloc_semaphore(f"gsem{i}") for i in range(NGATH)]
            nums = sorted(s.num for s in sems)
            assert nums == list(range(nums[0], nums[0] + NGATH))
            srange = range(nums[0], nums[-1] + 1)
            nc.gpsimd.dma_reset(srange)
            nc.gpsimd.sem_clear(srange)

            for rep in range(reps):
              for q in range(NQUART):
                # running max over the quarter: [p, ohi, (n, ch)]
                red = rp.tile([128, 2, TOKE], BF16, tag="red")
                redf = red[:].rearrange("p a b -> p (a b)")
                for h in range(GPQ):
                    gi = q * GPQ + h
                    g = gp.tile([128, RPG, ROWE], BF16, tag="g")
                    if stages != 'compute':
                        nc.gpsimd.dma_gather(
                            g[:], tab[:, :], idx_sb[:, gi, :],
                            NIDX, NIDX, ROWE,
                            single_packet=False,
                            queue_num=gi % nq,
                            prepare_only=True,
                            sem=sems[gi],
                        )
                        nc.gpsimd.trigger_dma(count=None, queue_num=gi % nq)
                        if rep == 0 and stages != 'nowait':
                            nc.vector.wait_ge(sems[gi], 16)
                    else:
                        nc.vector.memset(g[:, 0:1, 0:1], 0.0)
                    if stages == 'gather':
                        continue
                    # fold GROUP tokens within each row: [p, r, t, e]
                    g4 = g[:].rearrange("p r (t e) -> p r t e", t=GROUP)
                    t = GROUP
                    while t > 1:
                        nc.vector.tensor_tensor(
                            out=g4[:, :, 0:t // 2, :],
                            in0=g4[:, :, 0:t // 2, :],
                            in1=g4[:, :, t // 2:t, :], op=MAX)
                        t //= 2
                    # fold SPG slots: rows r = (slot, ohi)
                    w = g4[:, :, 0, :].rearrange("p (a b) e -> p a b e",
                                                 a=SPG)
                    a = SPG
                    while a > 1:
                        nc.vector.tensor_tensor(
                            out=w[:, 0:a // 2], in0=w[:, 0:a // 2],
                            in1=w[:, a // 2:a], op=MAX)
                        a //= 2
                    # w[:, 0] = [p, ohi, e] quarter partial (strided)
                    if h == 0:
                        nc.vector.tensor_copy(out=red[:], in_=w[:, 0])
                    else:
                        nc.vector.tensor_tensor(out=red[:], in0=red[:],
                                                in1=w[:, 0], op=MAX)

                if stages in ('gather', 'nogather_notail'):
                    continue
                # ---- tail for this quarter ----
                # red cols = (ohi, n, ch); transpose 128-col blocks
                # t = (ohi, nh) into rt[nh] cols (ohi, node).
                rts = [rtp.tile([128, 2, 128], BF16, tag=f"rt{nh}",
                                name=f"rt{nh}")
                       for nh in range(2)]
                for tb in range(4):
                    ohi, nh = tb // 2, tb % 2
                    pst = pstp.tile([128, 128], BF16, tag="pst")
                    nc.tensor.transpose(
                        out=pst[:],
                        in_=redf[:, tb * 128:(tb + 1) * 128],
                        identity=ident[:],
                    )
                    nc.vector.tensor_copy(out=rts[nh][:, ohi, :], in_=pst[:])

                # 2-batch block-diag matmuls: pair pi covers batches
                # (2*pi, 2*pi+1); rhs = rt[nh] flat [128, 256].
                for pi in range(4):
                    nh, bdi = pi // 2, pi % 2
                    pso = psmp.tile([128, QNODES], FP32, tag="pso")
                    nc.tensor.matmul(
                        out=pso[:],
                        lhsT=bd_sb[:, bdi, :],
                        rhs=rts[nh][:].rearrange("p a b -> p (a b)"),
                        start=True, stop=True,
                    )
                    osb = op.tile([128, QNODES], BF16, tag="osb")
                    nc.vector.tensor_tensor(
                        out=osb[:], in0=pso[:],
                        in1=bias_sb[:, q * QNODES:(q + 1) * QNODES],
                        op=mybir.AluOpType.add)
                    ne = 2 * pi
                    nc.sync.dma_start(
                        out=out[ne:ne + 2, :,
                                q * QNODES:(q + 1) * QNODES].rearrange(
                                    "a b c -> (a b) c"),
                        in_=osb[:])

    nc.compile()
    return nc


def _prep(x, nf_weight, ft_weight, bias, A):
    bf = ml_dtypes.bfloat16
    # token-major y rows: rows[j] = (x * nf)[:, :, j].ravel()
    y = x * nf_weight[None]
    rows = np.ascontiguousarray(y.transpose(2, 0, 1)).reshape(
        INN, TOKE).astype(bf)

    ftb = ft_weight.astype(bf)
    bdm = np.zeros((128, 2, 128), dtype=bf)
    bdm[0:32, 0, 0:64] = ftb
    bdm[32:64, 0, 64:128] = ftb
    bdm[64:96, 1, 0:64] = ftb
    bdm[96:128, 1, 64:128] = ftb

    in_maps = []
    for s in range(NCORES):
        A_s = np.sort(A[s * O_SH:(s + 1) * O_SH], axis=1)  # (1024, 32)
        grp = A_s.reshape(O_SH * NSLOT, GROUP)             # group rows
        uniq, inv = np.unique(grp, axis=0, return_inverse=True)
        assert len(uniq) <= VCAPG, len(uniq)
        # renumber table rows in gather-traversal (first-use) order so
        # each gather's HBM reads cluster in address space
        inv2 = inv.reshape(O_SH, NSLOT)
        trav = np.concatenate([
            inv2[(gi // GPQ) * QNODES:(gi // GPQ + 1) * QNODES,
                 (gi % GPQ) * SPG:(gi % GPQ + 1) * SPG].T.reshape(-1)
            for gi in range(NGATH)])
        first = np.full(len(uniq), -1, dtype=np.int64)
        order = []
        for r in trav:
            if first[r] < 0:
                first[r] = len(order)
                order.append(r)
        order = np.asarray(order)
        tab = np.zeros((VCAPG, ROWE), dtype=bf)
        tab[:len(uniq)] = rows[uniq[order]].reshape(len(uniq), ROWE)
        inv = first[inv]
        remap = inv.reshape(O_SH, NSLOT).astype(np.int16)  # [o_loc, slot]
        idx16 = np.zeros((NGATH, 128, NIDX // 16), dtype=np.int16)
        for gi in range(NGATH):
            q, h = gi // GPQ, gi % GPQ
            sub = remap[q * QNODES:(q + 1) * QNODES,
                        h * SPG:(h + 1) * SPG]    # [256 nodes, SPG slots]
            flat = sub.T.reshape(-1)              # j = slot*256 + o_loc
            idx16[gi, :16, :] = flat.reshape(NIDX // 16, 16).T
        idx16[:, 16:] = np.tile(idx16[:, :16], (1, 7, 1))
        bias_sh = bias[:, s * O_SH:(s + 1) * O_SH].astype(np.float32)
        in_maps.append({
            "tab": tab,
            "idx": idx16,
            "bd": bdm,
            "bias2": np.ascontiguousarray(np.tile(bias_sh, (2, 1))),
        })
    return in_maps


def run(x, nf_weight, ft_weight, bias, A, reps=1, stages='full', **run_kwargs):
    """Build (cached), run on 8 cores, reassemble. Returns (out, results)."""
    key = ("nc", reps, stages)
    if key not in _cache:
        _cache[key] = _build(reps, stages)
    nc = _cache[key]
    in_maps = _prep(np.asarray(x), np.asarray(nf_weight),
                    np.asarray(ft_weight), np.asarray(bias), np.asarray(A))
    res = run_bass_kernel_spmd(nc, in_maps, core_ids=list(range(NCORES)),
                               **run_kwargs)
    out = np.empty((N, OUTC, OUTN), dtype=np.float32)
    for s in range(NCORES):
        out[:, :, s * O_SH:(s + 1) * O_SH] = res.results[s]["out"].astype(
            np.float32)
    return out, res


def kernel(x, nf_weight, ft_weight, bias, A):
    out, _ = run(x, nf_weight, ft_weight, bias, A)
    return out
